# revision 8
# baseline (speedup 1.0000x reference)
"""DMAGLSTMCell Trainium2 kernel — data-parallel over batch on 8 NeuronCores.

Design (per core, batch shard of 8 rows):
  - All weights live in SBUF as bf16 (fp8 measured to give ZERO matmul
    speedup on this HW — per-matmul cost is ~54ns LDWEIGHTS-bound
    regardless of dtype — so bf16 everywhere for accuracy).
  - Activations flow transposed: PSUM [gate-dim-tile on partitions, batch
    on free]. Weight columns are packed chunk-major so each half of the
    units dim (chunk A = units 0:256, B = 256:512) has its 4 gates
    contiguous in one PSUM tile: [fsA flA alA oA | fsB flB alB oB | m |
    cbarA | cbarB] (single tile avoids per-instruction psum-switch cost;
    matmuls are grouped kc-inner so consecutive instructions hit the
    same psum slice — measured ~45ns/mm vs ~54ns otherwise).
  - Phase A precomputes gx[t] = x_t @ W_x + b for all t into DRAM; the
    loop injects it into PSUM via one identity-matmul.
  - 2-chunk cross-step software pipeline: h is produced in two halves
    (hA = units 0:256 first, then hB). The next step's matmuls are
    ordered kc01-block (needs hA only) then kc23-block (needs hB), so
    the PE starts step t+1 while step t's nonlinear tail (ACT/DVE) is
    still finishing chunk B. This hides the ~4.7us/step serial tail that
    dominated the unpipelined version.
  - Tail ops use fused scalar_tensor_tensor: with state c' = c+1 and
    S = sigmoid(2*cbar_pre): c' = f*(c'_prev - 2S) + 2S, and h is kept
    as h/2 = (sigmoid(2c'-2) - 0.5)*o with h-part weight rows pre-scaled
    by 2 (exact in bf16); the host rescales the output by 2.
  - hist copies run on the Pool engine to keep DVE for the tail chain.
"""
import sys
sys.path.insert(0, "/opt/trn_rl_repo")

import numpy as np
import ml_dtypes

BF16 = ml_dtypes.bfloat16

B, T, D, U = 64, 512, 256, 512
NC = 8            # cores
BS = B // NC      # batch shard per core = 8
KH = U // 128     # h-part contraction chunks = 4
KX = D // 128     # x-part contraction chunks = 2
MT_G = (4 * U + D) // 128   # gate m-tiles (fs,fl,alpha,o,m) = 18
MT_C = U // 128             # c-bar m-tiles = 4
MT = MT_G + MT_C            # 22
GF = MT_G * BS              # gates psum free width = 144
PF = MT * BS                # full psum free width = 176
WCOL = 2816                 # total output columns
TB = 64                     # phase-A t-block
NTB = T // TB               # 8
STG = TB * PF               # stage free size (gx slot incl b_C tail)
UNROLL = 8

# new column-block order (128-col blocks of W_all):
# [fsA fsA flA flA alA alA oA oA | fsB fsB flB flB alB alB oB oB | m m | C]
MT_PERM = [0, 1, 4, 5, 8, 9, 12, 13, 2, 3, 6, 7, 10, 11, 14, 15,
           16, 17, 18, 19, 20, 21]

_CACHE = {}


def _build_program(t_steps, loop_steps=None, rep=1, probe=None):
    import concourse.bass as bass
    import concourse.bacc as bacc
    import concourse.mybir as mybir
    from concourse import tile
    from concourse.bass import ds

    f32 = mybir.dt.float32
    bf16 = mybir.dt.bfloat16
    AF = mybir.ActivationFunctionType
    MUL = mybir.AluOpType.mult
    ADD = mybir.AluOpType.add

    if loop_steps is None:
        loop_steps = t_steps
    ntb = t_steps // TB
    nc = bacc.Bacc("TRN2", target_bir_lowering=False)

    # ---- DRAM I/O ----
    wsb_d = nc.dram_tensor("wsb", [128, 6 * WCOL], bf16, kind="ExternalInput")
    xt_d = nc.dram_tensor("xt", [128, KX * t_steps * BS], bf16, kind="ExternalInput")
    b22_d = nc.dram_tensor("b22", [128, MT], f32, kind="ExternalInput")
    bc64_d = nc.dram_tensor("bc64", [128, TB * MT_C * BS], bf16,
                            kind="ExternalInput")
    h0_d = nc.dram_tensor("h0p", [128, KH * BS], bf16, kind="ExternalInput")
    c0_d = nc.dram_tensor("c0p", [128, MT_C * BS], f32, kind="ExternalInput")
    eye_d = nc.dram_tensor("eye", [128, 128], bf16, kind="ExternalInput")
    ho_d = nc.dram_tensor("ho", [128, t_steps * KH * BS], f32, kind="ExternalOutput")
    gx_d = nc.dram_tensor("gxd", [128, t_steps * PF + 2 * UNROLL * PF], bf16,
                          kind="Internal")

    with tile.TileContext(nc) as tc:
        with (
            tc.tile_pool(name="persist", bufs=1) as pp,
            tc.tile_pool(name="stage", bufs=2) as sp,
            tc.tile_pool(name="scratch", bufs=2) as scp,
            tc.tile_pool(name="psA", bufs=2, space="PSUM") as ppA,
            tc.tile_pool(name="psS", bufs=2, space="PSUM") as pps,
        ):
            # ---- persistent SBUF ----
            wsb = pp.tile([128, 6 * WCOL], bf16)
            xt = pp.tile([128, KX * t_steps * BS], bf16)
            b22 = pp.tile([128, MT], f32)
            eye = pp.tile([128, 128], bf16)
            hist = pp.tile([128, (t_steps + 1) * KH * BS], bf16)
            cbuf = [pp.tile([128, MT_C * BS], f32, name=f"cst{i}", tag=f"c{i}")
                    for i in range(2)]
            gxb = [pp.tile([128, 4 * PF], bf16, name=f"gxb{i}",
                           tag=f"gx{i}") for i in range(2)]
            hpA = [pp.tile([128, 2 * BS], bf16, name=f"hpA{i}", tag=f"hA{i}")
                   for i in range(2)]
            hpB = [pp.tile([128, 2 * BS], bf16, name=f"hpB{i}", tag=f"hB{i}")
                   for i in range(2)]

            nc.sync.dma_start(wsb[:], wsb_d[:])
            nc.sync.dma_start(xt[:], xt_d[:])
            nc.sync.dma_start(b22[:], b22_d[:])
            nc.sync.dma_start(eye[:], eye_d[:])
            nc.sync.dma_start(hist[:, 0:KH * BS], h0_d[:])
            nc.sync.dma_start(hpA[0][:], h0_d[:, 0:2 * BS])
            nc.sync.dma_start(hpB[0][:], h0_d[:, 2 * BS:4 * BS])
            nc.sync.dma_start(cbuf[0][:], c0_d[:])

            def w_ap(kc, mt, ncols=128):
                return wsb[:, kc * WCOL + mt * 128: kc * WCOL + mt * 128 + ncols]

            # ---- Phase A: gx[t] = x_t @ W_x + b for all t ----
            for tb in range(ntb):
                stage = sp.tile([128, STG], bf16, tag="stage")
                st3 = stage[:].rearrange("p (t m) -> p t m", t=TB)
                for mt in range(MT_G):
                    ps = ppA.tile([128, TB * BS], f32, tag="psA")
                    for kc in range(KX):
                        rhs = xt[:, kc * t_steps * BS + tb * TB * BS:
                                 kc * t_steps * BS + (tb + 1) * TB * BS]
                        nc.tensor.matmul(ps[:], w_ap(4 + kc, mt), rhs,
                                         start=(kc == 0), stop=(kc == KX - 1))
                    ps3 = ps[:].rearrange("p (t b) -> p t b", t=TB)
                    nc.vector.tensor_scalar_add(
                        st3[:, :, mt * BS:(mt + 1) * BS], ps3, b22[:, mt:mt + 1])
                nc.sync.dma_start(
                    st3[:, :, GF:PF], bc64_d[:].rearrange(
                        "p (t m) -> p t m", t=TB))
                nc.sync.dma_start(gx_d[:, tb * STG:(tb + 1) * STG], stage[:])

            # zero the prefetch-overrun pad past the last real gx column
            negtwo = pp.tile([128, 1], f32)
            nc.vector.memset(negtwo[:], -2.0)
            zpad = pp.tile([128, 2 * UNROLL * PF], bf16)
            nc.vector.memset(zpad[:], 0.0)
            nc.sync.dma_start(
                gx_d[:, t_steps * PF:t_steps * PF + 2 * UNROLL * PF], zpad[:])

            # preload first two gx buffers (steps 0-3 / 4-7)
            half = 4 * PF
            nc.sync.dma_start(gxb[0][:], gx_d[:, 0:half])
            nc.sync.dma_start(gxb[1][:], gx_d[:, half:2 * half])

            # ---- recurrence (rep>1 only for timing experiments) ----
            with tc.For_i(0, rep, 1, hint_engines=(mybir.EngineType.PE,)):
              with tc.For_i(0, loop_steps, UNROLL,
                            hint_engines=(mybir.EngineType.PE,)) as iv:
                  for u in range(UNROLL):
                      buf = gxb[(u // 4) % 2]
                      ui = u % 4
                      cprev = cbuf[u % 2]
                      cnew = cbuf[(u + 1) % 2]
                      hA, hB = hpA[u % 2], hpB[u % 2]
                      hA2, hB2 = hpA[(u + 1) % 2], hpB[(u + 1) % 2]
                      ps = pps.tile([128, MT * BS], f32, tag="ps")

                      def hs(h, j):
                          return h[:, (j % 2) * BS:(j % 2) * BS + BS]

                      def mm(pst, lo, kc, mt, rhs, start=False, stop=False):
                          nc.tensor.matmul(
                              pst[:, (mt - lo) * BS:(mt - lo + 1) * BS],
                              w_ap(kc, mt), rhs, start=start, stop=stop,
                              skip_group_check=True)

                      # gx+bias inject (single identity matmul, one tile)
                      nc.tensor.matmul(ps[:], eye[:],
                                       buf[:, ui * PF:(ui + 1) * PF],
                                       start=True, stop=False,
                                       skip_group_check=True)

                      # Matmuls are grouped kc-inner so consecutive
                      # instructions hit the SAME psum slice: measured
                      # ~44ns/mm vs ~54ns when the out slice changes
                      # every instruction.
                      # Block-1: all kc0/kc1 matmuls (need hA only)
                      for mt in (16, 17):
                          for kc in (0, 1):
                              mm(ps, 0, kc, mt, hs(hA, kc))
                      for mt in range(0, 8):
                          for kc in (0, 1):
                              mm(ps, 0, kc, mt, hs(hA, kc))
                      for mt in (18, 19):
                          for kc in (0, 1):
                              mm(ps, 0, kc, mt, hs(hA, kc))
                      for mt in range(8, 16):
                          for kc in (0, 1):
                              mm(ps, 0, kc, mt, hs(hA, kc))
                      for mt in (20, 21):
                          for kc in (0, 1):
                              mm(ps, 0, kc, mt, hs(hA, kc))
                      # Block-2 (needs hB): m first -> Gm -> modx on
                      # ACT/DVE while PE sweeps gates-A
                      for mt in (16, 17):
                          for kc in (2, 3):
                              mm(ps, 0, kc, mt, hs(hB, kc), stop=(kc == 3))
                      Gm = scp.tile([128, KX * BS], bf16, tag="Gm")
                      nc.scalar.activation(Gm[:], ps[:, 16 * BS:18 * BS], AF.Sigmoid)
                      modx = scp.tile([128, KX * BS], bf16, tag="modx")
                      for kx in range(KX):
                          nc.vector.tensor_mul(
                              modx[:, kx * BS:(kx + 1) * BS],
                              Gm[:, kx * BS:(kx + 1) * BS],
                              xt[:, ds(kx * t_steps * BS + (iv + u) * BS, BS)])
                      # chunk-A matmuls complete first: gates-A, cbar-A + x
                      # (cbar mt runs are 4 long: kc2,kc3,kx0,kx1 same slice)
                      for mt in range(0, 8):
                          for kc in (2, 3):
                              mm(ps, 0, kc, mt, hs(hB, kc), stop=(kc == 3))
                      for mt in (18, 19):
                          for kc in (2, 3):
                              mm(ps, 0, kc, mt, hs(hB, kc))
                          for kx in range(KX):
                              mm(ps, 0, 4 + kx, mt,
                                 modx[:, kx * BS:(kx + 1) * BS],
                                 stop=(kx == KX - 1))
                      # chunk-A tail (ACT/DVE) — PE continues with B below
                      GA = scp.tile([128, 8 * BS], bf16, tag="GA")
                      nc.scalar.activation(GA[:], ps[:, 0:8 * BS], AF.Sigmoid)
                      SA = scp.tile([128, 2 * BS], f32, tag="SA")
                      nc.scalar.activation(SA[:], ps[:, 18 * BS:20 * BS],
                                           AF.Sigmoid, scale=2.0)
                      uuA = scp.tile([128, 2 * BS], bf16, tag="uuA")
                      wwA = scp.tile([128, 2 * BS], bf16, tag="wwA")
                      ffA = scp.tile([128, 2 * BS], f32, tag="ffA")
                      nc.vector.tensor_sub(uuA[:], GA[:, 0:16], GA[:, 16:32])
                      nc.vector.tensor_mul(wwA[:], GA[:, 32:48], uuA[:])
                      nc.vector.tensor_add(ffA[:], GA[:, 16:32], wwA[:])
                      rA = scp.tile([128, 2 * BS], f32, tag="rA")
                      nc.vector.scalar_tensor_tensor(
                          rA[:], SA[:], -2.0, cprev[:, 0:16], MUL, ADD)
                      tA = scp.tile([128, 2 * BS], f32, tag="tA")
                      nc.vector.tensor_mul(tA[:], ffA[:], rA[:])
                      nc.vector.scalar_tensor_tensor(
                          cnew[:, 0:16], SA[:], 2.0, tA[:], MUL, ADD)
                      S2A = scp.tile([128, 2 * BS], f32, tag="S2A")
                      nc.scalar.activation(S2A[:], cnew[:, 0:2 * BS],
                                           AF.Sigmoid, bias=negtwo[:],
                                           scale=2.0)
                      # hA' = (S2A - 0.5) * oA   (h stored as h/2)
                      nc.vector.scalar_tensor_tensor(
                          hA2[:], S2A[:], -0.5, GA[:, 48:64], ADD, MUL)

                      # chunk-B matmuls: gates-B, cbar-B + x
                      for mt in range(8, 16):
                          for kc in (2, 3):
                              mm(ps, 0, kc, mt, hs(hB, kc), stop=(kc == 3))
                      for mt in (20, 21):
                          for kc in (2, 3):
                              mm(ps, 0, kc, mt, hs(hB, kc))
                          for kx in range(KX):
                              mm(ps, 0, 4 + kx, mt,
                                 modx[:, kx * BS:(kx + 1) * BS],
                                 stop=(kx == KX - 1))
                      # chunk-B tail
                      GB = scp.tile([128, 8 * BS], bf16, tag="GB")
                      nc.scalar.activation(GB[:], ps[:, 8 * BS:16 * BS],
                                           AF.Sigmoid)
                      SB = scp.tile([128, 2 * BS], f32, tag="SB")
                      nc.scalar.activation(SB[:], ps[:, 20 * BS:22 * BS],
                                           AF.Sigmoid, scale=2.0)
                      uuB = scp.tile([128, 2 * BS], bf16, tag="uuB")
                      wwB = scp.tile([128, 2 * BS], bf16, tag="wwB")
                      ffB = scp.tile([128, 2 * BS], f32, tag="ffB")
                      nc.vector.tensor_sub(uuB[:], GB[:, 0:16], GB[:, 16:32])
                      nc.vector.tensor_mul(wwB[:], GB[:, 32:48], uuB[:])
                      nc.vector.tensor_add(ffB[:], GB[:, 16:32], wwB[:])
                      rB = scp.tile([128, 2 * BS], f32, tag="rB")
                      nc.vector.scalar_tensor_tensor(
                          rB[:], SB[:], -2.0, cprev[:, 16:32], MUL, ADD)
                      tB = scp.tile([128, 2 * BS], f32, tag="tB")
                      nc.vector.tensor_mul(tB[:], ffB[:], rB[:])
                      nc.vector.scalar_tensor_tensor(
                          cnew[:, 16:32], SB[:], 2.0, tB[:], MUL, ADD)
                      S2B = scp.tile([128, 2 * BS], f32, tag="S2B")
                      nc.scalar.activation(S2B[:], cnew[:, 2 * BS:4 * BS],
                                           AF.Sigmoid, bias=negtwo[:],
                                           scale=2.0)
                      nc.vector.scalar_tensor_tensor(
                          hB2[:], S2B[:], -0.5, GB[:, 48:64], ADD, MUL)
                      # hist copies on Pool engine (off the DVE chain)
                      nc.gpsimd.tensor_copy(
                          hist[:, ds((iv + u + 1) * KH * BS, 2 * BS)], hA2[:])
                      nc.gpsimd.tensor_copy(
                          hist[:, ds((iv + u + 1) * KH * BS + 2 * BS, 2 * BS)],
                          hB2[:])

                      # refill the just-drained gx half-buffer (4 steps ahead+1)
                      if u % 4 == 3:
                          nc.sync.dma_start(
                              gxb[(u // 4) % 2][:],
                              gx_d[:, ds((iv + u + 5) * PF, half)])

            # ---- output: cast history to fp32 ----
            nc.gpsimd.dma_start(ho_d[:], hist[:, KH * BS:(t_steps + 1) * KH * BS])

    nc.compile()
    return nc


def _pack_inputs(x, h0, c0, W_f_short, b_f_short, W_f_long, b_f_long,
                 W_alpha, b_alpha, W_m, b_m, W_C, b_C, W_o, b_o, t_steps):
    W_all = np.concatenate(
        [W_f_short, W_f_long, W_alpha, W_o, W_m, W_C], axis=1).astype(np.float32)
    b_all = np.concatenate(
        [b_f_short, b_f_long, b_alpha, b_o, b_m, b_C], axis=0).astype(np.float32)
    # permute 128-col blocks to the chunk-major layout
    W_all = W_all.reshape(D + U, MT, 128)[:, MT_PERM].reshape(D + U, WCOL)
    b_all = b_all.reshape(MT, 128)[MT_PERM].reshape(WCOL)
    # h stored as h/2 on device -> h-part weight rows x2 (exact in bf16)
    W_all[:U] *= 2.0
    # Wsb[p, kc*WCOL + m] = W_all[kc*128 + p, m]
    wsb = np.ascontiguousarray(
        W_all.reshape(6, 128, WCOL).transpose(1, 0, 2).reshape(128, 6 * WCOL)
    ).astype(BF16)
    b22 = np.ascontiguousarray(b_all.reshape(MT, 128).T).astype(np.float32)
    bc1 = np.ascontiguousarray(
        np.repeat(b_C.astype(np.float32).reshape(MT_C, 128).T[:, :, None],
                  BS, axis=2).reshape(128, MT_C * BS))
    bc64 = np.tile(bc1, (1, TB)).astype(BF16)
    eye = np.eye(128, dtype=np.float32).astype(BF16)

    ins = []
    for i in range(NC):
        xi = np.asarray(x[i * BS:(i + 1) * BS, :t_steps]).astype(np.float32)
        # xt[p, kc*T*BS + t*BS + b] = x[b, t, kc*128 + p]
        xti = np.ascontiguousarray(
            xi.reshape(BS, t_steps, KX, 128).transpose(3, 2, 1, 0)
            .reshape(128, KX * t_steps * BS)).astype(BF16)
        h0i = np.ascontiguousarray(
            (np.asarray(h0[i * BS:(i + 1) * BS]).astype(np.float32) * 0.5)
            .reshape(BS, KH, 128).transpose(2, 1, 0).reshape(128, KH * BS)
        ).astype(BF16)
        c0i = np.ascontiguousarray(
            (np.asarray(c0[i * BS:(i + 1) * BS]).astype(np.float32) + 1.0)
            .reshape(BS, MT_C, 128).transpose(2, 1, 0).reshape(128, MT_C * BS)
        ).astype(np.float32)
        ins.append({"wsb": wsb, "xt": xti, "b22": b22, "bc64": bc64,
                    "eye": eye, "h0p": h0i, "c0p": c0i})
    return ins


def kernel(**inputs):
    t_steps = int(np.asarray(inputs["x"]).shape[1])
    if t_steps not in _CACHE:
        _CACHE[t_steps] = _build_program(t_steps)
    nc = _CACHE[t_steps]

    from concourse.bass_utils import run_bass_kernel_spmd
    ins = _pack_inputs(t_steps=t_steps, **inputs)
    res = run_bass_kernel_spmd(nc, ins, core_ids=list(range(NC)))

    out = np.empty((B, t_steps, U), dtype=np.float32)
    for i in range(NC):
        ho = np.asarray(res.results[i]["ho"])  # [128, T*KH*BS]
        a = ho.reshape(128, t_steps, KH, BS)
        # stored h/2 -> rescale by 2 (exact)
        out[i * BS:(i + 1) * BS] = (
            a.transpose(3, 1, 2, 0).reshape(BS, t_steps, U) * 2.0)
    return out


if __name__ == "__main__":
    rng = np.random.default_rng(0)
    sh = {"x": (B, T, D), "h0": (B, U), "c0": (B, U)}
    demo = {k: rng.standard_normal(v).astype(np.float32) * 0.1
            for k, v in sh.items()}
    for n, s in [("W_f_short", (D + U, U)), ("W_f_long", (D + U, U)),
                 ("W_alpha", (D + U, U)), ("W_m", (D + U, D)),
                 ("W_C", (D + U, U)), ("W_o", (D + U, U))]:
        demo[n] = rng.standard_normal(s).astype(np.float32) * 0.05
    for n, s in [("b_f_short", U), ("b_f_long", U), ("b_alpha", U),
                 ("b_m", D), ("b_C", U), ("b_o", U)]:
        demo[n] = np.zeros(s, np.float32)
    out = kernel(**demo)
    print(out.shape, out.dtype)


# revision 9
# speedup vs baseline: 1.2238x; 1.2238x over previous
"""DMAGLSTMCell Trainium2 kernel — data-parallel over batch on 8 NeuronCores.

Design (per core, batch shard of 8 rows):
  - All weights live in SBUF as bf16 (fp8 measured to give ZERO matmul
    speedup on this HW — per-matmul cost is ~54ns LDWEIGHTS-bound
    regardless of dtype — so bf16 everywhere for accuracy).
  - Activations flow transposed: PSUM [gate-dim-tile on partitions, batch
    on free]. Weight columns are packed chunk-major so each half of the
    units dim (chunk A = units 0:256, B = 256:512) has its 4 gates
    contiguous in PSUM: psG free = [fsA flA alA oA | fsB flB alB oB],
    psM free = [m | cbarA | cbarB].
  - Phase A precomputes gx[t] = x_t @ W_x + b for all t into DRAM; the
    loop injects it into PSUM via one identity-matmul per PSUM bank.
  - 2-chunk cross-step software pipeline: h is produced in two halves
    (hA = units 0:256 first, then hB). The next step's matmuls are
    ordered kc01-block (needs hA only) then kc23-block (needs hB), so
    the PE starts step t+1 while step t's nonlinear tail (ACT/DVE) is
    still finishing chunk B. This hides the ~4.7us/step serial tail that
    dominated the unpipelined version.
  - Tail ops use fused scalar_tensor_tensor: with state c' = c+1 and
    S = sigmoid(2*cbar_pre): c' = f*(c'_prev - 2S) + 2S, and h is kept
    as h/2 = (sigmoid(2c'-2) - 0.5)*o with h-part weight rows pre-scaled
    by 2 (exact in bf16); the host rescales the output by 2.
  - hist copies run on the Pool engine to keep DVE for the tail chain.
"""
import sys
sys.path.insert(0, "/opt/trn_rl_repo")

import numpy as np
import ml_dtypes

BF16 = ml_dtypes.bfloat16

B, T, D, U = 64, 512, 256, 512
NC = 8            # cores
BS = B // NC      # batch shard per core = 8
KH = U // 128     # h-part contraction chunks = 4
KX = D // 128     # x-part contraction chunks = 2
MT_G = (4 * U + D) // 128   # gate m-tiles (fs,fl,alpha,o,m) = 18
MT_C = U // 128             # c-bar m-tiles = 4
MT = MT_G + MT_C            # 22
GF = MT_G * BS              # gates psum free width = 144
PF = MT * BS                # full psum free width = 176
WCOL = 2816                 # total output columns
TB = 64                     # phase-A t-block
NTB = T // TB               # 8
STG = TB * PF               # stage free size (gx slot incl b_C tail)
UNROLL = 8

# new column-block order (128-col blocks of W_all):
# [fsA fsA flA flA alA alA oA oA | fsB fsB flB flB alB alB oB oB | m m | C]
MT_PERM = [0, 1, 4, 5, 8, 9, 12, 13, 2, 3, 6, 7, 10, 11, 14, 15,
           16, 17, 18, 19, 20, 21]

_CACHE = {}


def _build_program(t_steps, loop_steps=None, rep=1, probe=None):
    import concourse.bass as bass
    import concourse.bacc as bacc
    import concourse.mybir as mybir
    from concourse import tile
    from concourse.bass import ds

    f32 = mybir.dt.float32
    bf16 = mybir.dt.bfloat16
    AF = mybir.ActivationFunctionType
    MUL = mybir.AluOpType.mult
    ADD = mybir.AluOpType.add

    if loop_steps is None:
        loop_steps = t_steps
    ntb = t_steps // TB
    nc = bacc.Bacc("TRN2", target_bir_lowering=False)

    # ---- DRAM I/O ----
    wsb_d = nc.dram_tensor("wsb", [128, 6 * WCOL], bf16, kind="ExternalInput")
    xt_d = nc.dram_tensor("xt", [128, KX * t_steps * BS], bf16, kind="ExternalInput")
    b22_d = nc.dram_tensor("b22", [128, MT], f32, kind="ExternalInput")
    bc64_d = nc.dram_tensor("bc64", [128, TB * MT_C * BS], bf16,
                            kind="ExternalInput")
    h0_d = nc.dram_tensor("h0p", [128, KH * BS], bf16, kind="ExternalInput")
    c0_d = nc.dram_tensor("c0p", [128, MT_C * BS], f32, kind="ExternalInput")
    eye_d = nc.dram_tensor("eye", [128, 128], bf16, kind="ExternalInput")
    ho_d = nc.dram_tensor("ho", [128, t_steps * KH * BS], f32, kind="ExternalOutput")
    gx_d = nc.dram_tensor("gxd", [128, t_steps * PF + 2 * UNROLL * PF], bf16,
                          kind="Internal")

    with tile.TileContext(nc) as tc:
        with (
            tc.tile_pool(name="persist", bufs=1) as pp,
            tc.tile_pool(name="stage", bufs=2) as sp,
            tc.tile_pool(name="scratch", bufs=2) as scp,
            tc.tile_pool(name="psA", bufs=2, space="PSUM") as ppA,
            tc.tile_pool(name="psG", bufs=2, space="PSUM") as ppG,
            tc.tile_pool(name="psM", bufs=2, space="PSUM") as ppM,
        ):
            # ---- persistent SBUF ----
            wsb = pp.tile([128, 6 * WCOL], bf16)
            xt = pp.tile([128, KX * t_steps * BS], bf16)
            b22 = pp.tile([128, MT], f32)
            eye = pp.tile([128, 128], bf16)
            hist = pp.tile([128, (t_steps + 1) * KH * BS], bf16)
            cbuf = [pp.tile([128, MT_C * BS], f32, name=f"cst{i}", tag=f"c{i}")
                    for i in range(2)]
            gxb = [pp.tile([128, 4 * PF], bf16, name=f"gxb{i}",
                           tag=f"gx{i}") for i in range(2)]
            hpA = [pp.tile([128, 2 * BS], bf16, name=f"hpA{i}", tag=f"hA{i}")
                   for i in range(2)]
            hpB = [pp.tile([128, 2 * BS], bf16, name=f"hpB{i}", tag=f"hB{i}")
                   for i in range(2)]

            nc.sync.dma_start(wsb[:], wsb_d[:])
            nc.sync.dma_start(xt[:], xt_d[:])
            nc.sync.dma_start(b22[:], b22_d[:])
            nc.sync.dma_start(eye[:], eye_d[:])
            nc.sync.dma_start(hist[:, 0:KH * BS], h0_d[:])
            nc.sync.dma_start(hpA[0][:], h0_d[:, 0:2 * BS])
            nc.sync.dma_start(hpB[0][:], h0_d[:, 2 * BS:4 * BS])
            nc.sync.dma_start(cbuf[0][:], c0_d[:])

            def w_ap(kc, mt, ncols=128):
                return wsb[:, kc * WCOL + mt * 128: kc * WCOL + mt * 128 + ncols]

            # ---- Phase A: gx[t] = x_t @ W_x + b for all t ----
            for tb in range(ntb):
                stage = sp.tile([128, STG], bf16, tag="stage")
                st3 = stage[:].rearrange("p (t m) -> p t m", t=TB)
                for mt in range(MT_G):
                    ps = ppA.tile([128, TB * BS], f32, tag="psA")
                    for kc in range(KX):
                        rhs = xt[:, kc * t_steps * BS + tb * TB * BS:
                                 kc * t_steps * BS + (tb + 1) * TB * BS]
                        nc.tensor.matmul(ps[:], w_ap(4 + kc, mt), rhs,
                                         start=(kc == 0), stop=(kc == KX - 1))
                    ps3 = ps[:].rearrange("p (t b) -> p t b", t=TB)
                    nc.vector.tensor_scalar_add(
                        st3[:, :, mt * BS:(mt + 1) * BS], ps3, b22[:, mt:mt + 1])
                nc.sync.dma_start(
                    st3[:, :, GF:PF], bc64_d[:].rearrange(
                        "p (t m) -> p t m", t=TB))
                nc.sync.dma_start(gx_d[:, tb * STG:(tb + 1) * STG], stage[:])

            # zero the prefetch-overrun pad past the last real gx column
            negtwo = pp.tile([128, 1], f32)
            nc.vector.memset(negtwo[:], -2.0)
            zpad = pp.tile([128, 2 * UNROLL * PF], bf16)
            nc.vector.memset(zpad[:], 0.0)
            nc.sync.dma_start(
                gx_d[:, t_steps * PF:t_steps * PF + 2 * UNROLL * PF], zpad[:])

            # preload first two gx buffers (steps 0-3 / 4-7)
            half = 4 * PF
            nc.sync.dma_start(gxb[0][:], gx_d[:, 0:half])
            nc.sync.dma_start(gxb[1][:], gx_d[:, half:2 * half])

            # ---- recurrence (rep>1 only for timing experiments) ----
            with tc.For_i(0, rep, 1, hint_engines=(mybir.EngineType.PE,)):
              with tc.For_i(0, loop_steps, UNROLL,
                            hint_engines=(mybir.EngineType.PE,)) as iv:
                  for u in range(UNROLL):
                      buf = gxb[(u // 4) % 2]
                      ui = u % 4
                      cprev = cbuf[u % 2]
                      cnew = cbuf[(u + 1) % 2]
                      hA, hB = hpA[u % 2], hpB[u % 2]
                      hA2, hB2 = hpA[(u + 1) % 2], hpB[(u + 1) % 2]
                      psG = ppG.tile([128, 16 * BS], f32, tag="psG")
                      psM = ppM.tile([128, 6 * BS], f32, tag="psM")

                      def hs(h, j):
                          return h[:, (j % 2) * BS:(j % 2) * BS + BS]

                      def mm(pst, lo, kc, mt, rhs, start=False, stop=False):
                          nc.tensor.matmul(
                              pst[:, (mt - lo) * BS:(mt - lo + 1) * BS],
                              w_ap(kc, mt), rhs, start=start, stop=stop,
                              skip_group_check=True)

                      # gx+bias inject (identity matmuls, one per PSUM bank)
                      nc.tensor.matmul(psG[:], eye[:],
                                       buf[:, ui * PF:ui * PF + 128],
                                       start=True, stop=False,
                                       skip_group_check=True)
                      nc.tensor.matmul(psM[:], eye[:],
                                       buf[:, ui * PF + 128:(ui + 1) * PF],
                                       start=True, stop=False,
                                       skip_group_check=True)

                      # Block-1: all kc0/kc1 matmuls (need hA only)
                      for kc in (0, 1):
                          rhs = hs(hA, kc)
                          for mt in (16, 17):
                              mm(psM, 16, kc, mt, rhs)
                          for mt in range(0, 8):
                              mm(psG, 0, kc, mt, rhs)
                          for mt in (18, 19):
                              mm(psM, 16, kc, mt, rhs)
                          for mt in range(8, 16):
                              mm(psG, 0, kc, mt, rhs)
                          for mt in (20, 21):
                              mm(psM, 16, kc, mt, rhs)
                      # Block-2 (needs hB): m first -> Gm -> modx on
                      # ACT/DVE while PE sweeps gates-A
                      for kc in (2, 3):
                          for mt in (16, 17):
                              mm(psM, 16, kc, mt, hs(hB, kc), stop=(kc == 3))
                      Gm = scp.tile([128, KX * BS], bf16, tag="Gm")
                      nc.scalar.activation(Gm[:], psM[:, 0:2 * BS], AF.Sigmoid)
                      modx = scp.tile([128, KX * BS], bf16, tag="modx")
                      for kx in range(KX):
                          nc.vector.tensor_mul(
                              modx[:, kx * BS:(kx + 1) * BS],
                              Gm[:, kx * BS:(kx + 1) * BS],
                              xt[:, ds(kx * t_steps * BS + (iv + u) * BS, BS)])
                      # chunk-A matmuls complete first: gates-A, cbar-A + x
                      for kc in (2, 3):
                          rhs = hs(hB, kc)
                          for mt in range(0, 8):
                              mm(psG, 0, kc, mt, rhs, stop=(kc == 3))
                          for mt in (18, 19):
                              mm(psM, 16, kc, mt, rhs)
                      for kx in range(KX):
                          for mt in (18, 19):
                              mm(psM, 16, 4 + kx, mt,
                                 modx[:, kx * BS:(kx + 1) * BS],
                                 stop=(kx == KX - 1))
                      # chunk-A tail (ACT/DVE) — PE continues with B below
                      GA = scp.tile([128, 8 * BS], bf16, tag="GA")
                      nc.scalar.activation(GA[:], psG[:, 0:8 * BS], AF.Sigmoid)
                      SA = scp.tile([128, 2 * BS], f32, tag="SA")
                      nc.scalar.activation(SA[:], psM[:, 2 * BS:4 * BS],
                                           AF.Sigmoid, scale=2.0)
                      uuA = scp.tile([128, 2 * BS], bf16, tag="uuA")
                      wwA = scp.tile([128, 2 * BS], bf16, tag="wwA")
                      ffA = scp.tile([128, 2 * BS], f32, tag="ffA")
                      nc.vector.tensor_sub(uuA[:], GA[:, 0:16], GA[:, 16:32])
                      nc.vector.tensor_mul(wwA[:], GA[:, 32:48], uuA[:])
                      nc.vector.tensor_add(ffA[:], GA[:, 16:32], wwA[:])
                      rA = scp.tile([128, 2 * BS], f32, tag="rA")
                      nc.vector.scalar_tensor_tensor(
                          rA[:], SA[:], -2.0, cprev[:, 0:16], MUL, ADD)
                      tA = scp.tile([128, 2 * BS], f32, tag="tA")
                      nc.vector.tensor_mul(tA[:], ffA[:], rA[:])
                      nc.vector.scalar_tensor_tensor(
                          cnew[:, 0:16], SA[:], 2.0, tA[:], MUL, ADD)
                      S2A = scp.tile([128, 2 * BS], f32, tag="S2A")
                      nc.scalar.activation(S2A[:], cnew[:, 0:2 * BS],
                                           AF.Sigmoid, bias=negtwo[:],
                                           scale=2.0)
                      # hA' = (S2A - 0.5) * oA   (h stored as h/2)
                      nc.vector.scalar_tensor_tensor(
                          hA2[:], S2A[:], -0.5, GA[:, 48:64], ADD, MUL)

                      # chunk-B matmuls: gates-B, cbar-B + x
                      for kc in (2, 3):
                          rhs = hs(hB, kc)
                          for mt in range(8, 16):
                              mm(psG, 0, kc, mt, rhs, stop=(kc == 3))
                          for mt in (20, 21):
                              mm(psM, 16, kc, mt, rhs)
                      for kx in range(KX):
                          for mt in (20, 21):
                              mm(psM, 16, 4 + kx, mt,
                                 modx[:, kx * BS:(kx + 1) * BS],
                                 stop=(kx == KX - 1))
                      # chunk-B tail
                      GB = scp.tile([128, 8 * BS], bf16, tag="GB")
                      nc.scalar.activation(GB[:], psG[:, 8 * BS:16 * BS],
                                           AF.Sigmoid)
                      SB = scp.tile([128, 2 * BS], f32, tag="SB")
                      nc.scalar.activation(SB[:], psM[:, 4 * BS:6 * BS],
                                           AF.Sigmoid, scale=2.0)
                      uuB = scp.tile([128, 2 * BS], bf16, tag="uuB")
                      wwB = scp.tile([128, 2 * BS], bf16, tag="wwB")
                      ffB = scp.tile([128, 2 * BS], f32, tag="ffB")
                      nc.vector.tensor_sub(uuB[:], GB[:, 0:16], GB[:, 16:32])
                      nc.vector.tensor_mul(wwB[:], GB[:, 32:48], uuB[:])
                      nc.vector.tensor_add(ffB[:], GB[:, 16:32], wwB[:])
                      rB = scp.tile([128, 2 * BS], f32, tag="rB")
                      nc.vector.scalar_tensor_tensor(
                          rB[:], SB[:], -2.0, cprev[:, 16:32], MUL, ADD)
                      tB = scp.tile([128, 2 * BS], f32, tag="tB")
                      nc.vector.tensor_mul(tB[:], ffB[:], rB[:])
                      nc.vector.scalar_tensor_tensor(
                          cnew[:, 16:32], SB[:], 2.0, tB[:], MUL, ADD)
                      S2B = scp.tile([128, 2 * BS], f32, tag="S2B")
                      nc.scalar.activation(S2B[:], cnew[:, 2 * BS:4 * BS],
                                           AF.Sigmoid, bias=negtwo[:],
                                           scale=2.0)
                      nc.vector.scalar_tensor_tensor(
                          hB2[:], S2B[:], -0.5, GB[:, 48:64], ADD, MUL)
                      # hist copies on Pool engine (off the DVE chain)
                      nc.gpsimd.tensor_copy(
                          hist[:, ds((iv + u + 1) * KH * BS, 2 * BS)], hA2[:])
                      nc.gpsimd.tensor_copy(
                          hist[:, ds((iv + u + 1) * KH * BS + 2 * BS, 2 * BS)],
                          hB2[:])

                      # refill the just-drained gx half-buffer (4 steps ahead+1)
                      if u % 4 == 3:
                          nc.sync.dma_start(
                              gxb[(u // 4) % 2][:],
                              gx_d[:, ds((iv + u + 5) * PF, half)])

            # ---- output: cast history to fp32 ----
            nc.gpsimd.dma_start(ho_d[:], hist[:, KH * BS:(t_steps + 1) * KH * BS])

    nc.compile()
    return nc


def _pack_inputs(x, h0, c0, W_f_short, b_f_short, W_f_long, b_f_long,
                 W_alpha, b_alpha, W_m, b_m, W_C, b_C, W_o, b_o, t_steps):
    W_all = np.concatenate(
        [W_f_short, W_f_long, W_alpha, W_o, W_m, W_C], axis=1).astype(np.float32)
    b_all = np.concatenate(
        [b_f_short, b_f_long, b_alpha, b_o, b_m, b_C], axis=0).astype(np.float32)
    # permute 128-col blocks to the chunk-major layout
    W_all = W_all.reshape(D + U, MT, 128)[:, MT_PERM].reshape(D + U, WCOL)
    b_all = b_all.reshape(MT, 128)[MT_PERM].reshape(WCOL)
    # h stored as h/2 on device -> h-part weight rows x2 (exact in bf16)
    W_all[:U] *= 2.0
    # Wsb[p, kc*WCOL + m] = W_all[kc*128 + p, m]
    wsb = np.ascontiguousarray(
        W_all.reshape(6, 128, WCOL).transpose(1, 0, 2).reshape(128, 6 * WCOL)
    ).astype(BF16)
    b22 = np.ascontiguousarray(b_all.reshape(MT, 128).T).astype(np.float32)
    bc1 = np.ascontiguousarray(
        np.repeat(b_C.astype(np.float32).reshape(MT_C, 128).T[:, :, None],
                  BS, axis=2).reshape(128, MT_C * BS))
    bc64 = np.tile(bc1, (1, TB)).astype(BF16)
    eye = np.eye(128, dtype=np.float32).astype(BF16)

    ins = []
    for i in range(NC):
        xi = np.asarray(x[i * BS:(i + 1) * BS, :t_steps]).astype(np.float32)
        # xt[p, kc*T*BS + t*BS + b] = x[b, t, kc*128 + p]
        xti = np.ascontiguousarray(
            xi.reshape(BS, t_steps, KX, 128).transpose(3, 2, 1, 0)
            .reshape(128, KX * t_steps * BS)).astype(BF16)
        h0i = np.ascontiguousarray(
            (np.asarray(h0[i * BS:(i + 1) * BS]).astype(np.float32) * 0.5)
            .reshape(BS, KH, 128).transpose(2, 1, 0).reshape(128, KH * BS)
        ).astype(BF16)
        c0i = np.ascontiguousarray(
            (np.asarray(c0[i * BS:(i + 1) * BS]).astype(np.float32) + 1.0)
            .reshape(BS, MT_C, 128).transpose(2, 1, 0).reshape(128, MT_C * BS)
        ).astype(np.float32)
        ins.append({"wsb": wsb, "xt": xti, "b22": b22, "bc64": bc64,
                    "eye": eye, "h0p": h0i, "c0p": c0i})
    return ins


def kernel(**inputs):
    t_steps = int(np.asarray(inputs["x"]).shape[1])
    if t_steps not in _CACHE:
        _CACHE[t_steps] = _build_program(t_steps)
    nc = _CACHE[t_steps]

    from concourse.bass_utils import run_bass_kernel_spmd
    ins = _pack_inputs(t_steps=t_steps, **inputs)
    res = run_bass_kernel_spmd(nc, ins, core_ids=list(range(NC)))

    out = np.empty((B, t_steps, U), dtype=np.float32)
    for i in range(NC):
        ho = np.asarray(res.results[i]["ho"])  # [128, T*KH*BS]
        a = ho.reshape(128, t_steps, KH, BS)
        # stored h/2 -> rescale by 2 (exact)
        out[i * BS:(i + 1) * BS] = (
            a.transpose(3, 1, 2, 0).reshape(BS, t_steps, U) * 2.0)
    return out


if __name__ == "__main__":
    rng = np.random.default_rng(0)
    sh = {"x": (B, T, D), "h0": (B, U), "c0": (B, U)}
    demo = {k: rng.standard_normal(v).astype(np.float32) * 0.1
            for k, v in sh.items()}
    for n, s in [("W_f_short", (D + U, U)), ("W_f_long", (D + U, U)),
                 ("W_alpha", (D + U, U)), ("W_m", (D + U, D)),
                 ("W_C", (D + U, U)), ("W_o", (D + U, U))]:
        demo[n] = rng.standard_normal(s).astype(np.float32) * 0.05
    for n, s in [("b_f_short", U), ("b_f_long", U), ("b_alpha", U),
                 ("b_m", D), ("b_C", U), ("b_o", U)]:
        demo[n] = np.zeros(s, np.float32)
    out = kernel(**demo)
    print(out.shape, out.dtype)


# revision 10
# speedup vs baseline: 1.2708x; 1.0384x over previous
"""DMAGLSTMCell Trainium2 kernel — data-parallel over batch on 8 NeuronCores.

Design (per core, batch shard of 8 rows):
  - All weights live in SBUF as bf16 (fp8 measured to give ZERO matmul
    speedup on this HW — per-matmul cost is ~54ns LDWEIGHTS-bound
    regardless of dtype — so bf16 everywhere for accuracy).
  - Activations flow transposed: PSUM [gate-dim-tile on partitions, batch
    on free]. Weight columns are packed chunk-major so each half of the
    units dim (chunk A = units 0:256, B = 256:512) has its 4 gates
    contiguous in PSUM: psG free = [fsA flA alA oA | fsB flB alB oB],
    psM free = [m | cbarA | cbarB].
  - Phase A precomputes gx[t] = x_t @ W_x + b for all t into DRAM; the
    loop injects it into PSUM via one identity-matmul per PSUM bank.
  - 2-chunk cross-step software pipeline: h is produced in two halves
    (hA = units 0:256 first, then hB). The next step's matmuls are
    ordered kc01-block (needs hA only) then kc23-block (needs hB), so
    the PE starts step t+1 while step t's nonlinear tail (ACT/DVE) is
    still finishing chunk B. This hides the ~4.7us/step serial tail that
    dominated the unpipelined version.
  - Tail ops use fused scalar_tensor_tensor: with state c' = c+1 and
    S = sigmoid(2*cbar_pre): c' = f*(c'_prev - 2S) + 2S, and h is kept
    as h/2 = (sigmoid(2c'-2) - 0.5)*o with h-part weight rows pre-scaled
    by 2 (exact in bf16); the host rescales the output by 2.
  - hist copies run on the Pool engine to keep DVE for the tail chain.
"""
import sys
sys.path.insert(0, "/opt/trn_rl_repo")

import numpy as np
import ml_dtypes

BF16 = ml_dtypes.bfloat16

B, T, D, U = 64, 512, 256, 512
NC = 8            # cores
BS = B // NC      # batch shard per core = 8
KH = U // 128     # h-part contraction chunks = 4
KX = D // 128     # x-part contraction chunks = 2
MT_G = (4 * U + D) // 128   # gate m-tiles (fs,fl,alpha,o,m) = 18
MT_C = U // 128             # c-bar m-tiles = 4
MT = MT_G + MT_C            # 22
GF = MT_G * BS              # gates psum free width = 144
PF = MT * BS                # full psum free width = 176
WCOL = 2816                 # total output columns
TB = 64                     # phase-A t-block
NTB = T // TB               # 8
STG = TB * PF               # stage free size (gx slot incl b_C tail)
UNROLL = 8

# new column-block order (128-col blocks of W_all):
# [fsA fsA flA flA alA alA oA oA | fsB fsB flB flB alB alB oB oB | m m | C]
MT_PERM = [0, 1, 4, 5, 8, 9, 12, 13, 2, 3, 6, 7, 10, 11, 14, 15,
           16, 17, 18, 19, 20, 21]

_CACHE = {}


def _build_program(t_steps, loop_steps=None, rep=1, probe=None):
    import concourse.bass as bass
    import concourse.bacc as bacc
    import concourse.mybir as mybir
    from concourse import tile
    from concourse.bass import ds

    f32 = mybir.dt.float32
    bf16 = mybir.dt.bfloat16
    AF = mybir.ActivationFunctionType
    MUL = mybir.AluOpType.mult
    ADD = mybir.AluOpType.add

    if loop_steps is None:
        loop_steps = t_steps
    ntb = t_steps // TB
    nc = bacc.Bacc("TRN2", target_bir_lowering=False)

    # ---- DRAM I/O ----
    wsb_d = nc.dram_tensor("wsb", [128, 6 * WCOL], bf16, kind="ExternalInput")
    xt_d = nc.dram_tensor("xt", [128, KX * t_steps * BS], bf16, kind="ExternalInput")
    b22_d = nc.dram_tensor("b22", [128, MT], f32, kind="ExternalInput")
    bc64_d = nc.dram_tensor("bc64", [128, TB * MT_C * BS], bf16,
                            kind="ExternalInput")
    h0_d = nc.dram_tensor("h0p", [128, KH * BS], bf16, kind="ExternalInput")
    c0_d = nc.dram_tensor("c0p", [128, MT_C * BS], f32, kind="ExternalInput")
    eye_d = nc.dram_tensor("eye", [128, 128], bf16, kind="ExternalInput")
    w8_d = nc.dram_tensor("wsb8", [128, 4 * 16 * 128], mybir.dt.float8e4,
                          kind="ExternalInput")
    ho_d = nc.dram_tensor("ho", [128, t_steps * KH * BS], f32, kind="ExternalOutput")
    gx_d = nc.dram_tensor("gxd", [128, t_steps * PF + 2 * UNROLL * PF], bf16,
                          kind="Internal")

    with tile.TileContext(nc) as tc:
        with (
            tc.tile_pool(name="persist", bufs=1) as pp,
            tc.tile_pool(name="stage", bufs=2) as sp,
            tc.tile_pool(name="scratch", bufs=2) as scp,
            tc.tile_pool(name="psA", bufs=2, space="PSUM") as ppA,
            tc.tile_pool(name="psG", bufs=2, space="PSUM") as ppG,
            tc.tile_pool(name="psM", bufs=2, space="PSUM") as ppM,
        ):
            # ---- persistent SBUF ----
            wsb = pp.tile([128, 6 * WCOL], bf16)
            xt = pp.tile([128, KX * t_steps * BS], bf16)
            b22 = pp.tile([128, MT], f32)
            eye = pp.tile([128, 128], bf16)
            wsb8 = pp.tile([128, 4 * 16 * 128], mybir.dt.float8e4)
            hist = pp.tile([128, (t_steps + 1) * KH * BS], bf16)
            cbuf = [pp.tile([128, MT_C * BS], f32, name=f"cst{i}", tag=f"c{i}")
                    for i in range(2)]
            gxb = [pp.tile([128, 4 * PF], bf16, name=f"gxb{i}",
                           tag=f"gx{i}") for i in range(2)]
            hpA = [pp.tile([128, 2 * BS], bf16, name=f"hpA{i}", tag=f"hA{i}")
                   for i in range(2)]
            hpB = [pp.tile([128, 2 * BS], bf16, name=f"hpB{i}", tag=f"hB{i}")
                   for i in range(2)]

            nc.sync.dma_start(wsb[:], wsb_d[:])
            nc.sync.dma_start(xt[:], xt_d[:])
            nc.sync.dma_start(b22[:], b22_d[:])
            nc.sync.dma_start(eye[:], eye_d[:])
            nc.sync.dma_start(wsb8[:], w8_d[:])
            nc.sync.dma_start(hist[:, 0:KH * BS], h0_d[:])
            nc.sync.dma_start(hpA[0][:], h0_d[:, 0:2 * BS])
            nc.sync.dma_start(hpB[0][:], h0_d[:, 2 * BS:4 * BS])
            nc.sync.dma_start(cbuf[0][:], c0_d[:])

            def w_ap(kc, mt, ncols=128):
                return wsb[:, kc * WCOL + mt * 128: kc * WCOL + mt * 128 + ncols]

            # ---- Phase A: gx[t] = x_t @ W_x + b for all t ----
            for tb in range(ntb):
                stage = sp.tile([128, STG], bf16, tag="stage")
                st3 = stage[:].rearrange("p (t m) -> p t m", t=TB)
                for mt in range(MT_G):
                    ps = ppA.tile([128, TB * BS], f32, tag="psA")
                    for kc in range(KX):
                        rhs = xt[:, kc * t_steps * BS + tb * TB * BS:
                                 kc * t_steps * BS + (tb + 1) * TB * BS]
                        nc.tensor.matmul(ps[:], w_ap(4 + kc, mt), rhs,
                                         start=(kc == 0), stop=(kc == KX - 1))
                    ps3 = ps[:].rearrange("p (t b) -> p t b", t=TB)
                    nc.vector.tensor_scalar_add(
                        st3[:, :, mt * BS:(mt + 1) * BS], ps3, b22[:, mt:mt + 1])
                nc.sync.dma_start(
                    st3[:, :, GF:PF], bc64_d[:].rearrange(
                        "p (t m) -> p t m", t=TB))
                nc.sync.dma_start(gx_d[:, tb * STG:(tb + 1) * STG], stage[:])

            # zero the prefetch-overrun pad past the last real gx column
            negtwo = pp.tile([128, 1], f32)
            nc.vector.memset(negtwo[:], -2.0)
            zpad = pp.tile([128, 2 * UNROLL * PF], bf16)
            nc.vector.memset(zpad[:], 0.0)
            nc.sync.dma_start(
                gx_d[:, t_steps * PF:t_steps * PF + 2 * UNROLL * PF], zpad[:])

            # preload first two gx buffers (steps 0-3 / 4-7)
            half = 4 * PF
            nc.sync.dma_start(gxb[0][:], gx_d[:, 0:half])
            nc.sync.dma_start(gxb[1][:], gx_d[:, half:2 * half])

            # ---- recurrence (rep>1 only for timing experiments) ----
            with tc.For_i(0, rep, 1, hint_engines=(mybir.EngineType.PE,)):
              with tc.For_i(0, loop_steps, UNROLL,
                            hint_engines=(mybir.EngineType.PE,)) as iv:
                  for u in range(UNROLL):
                      buf = gxb[(u // 4) % 2]
                      ui = u % 4
                      cprev = cbuf[u % 2]
                      cnew = cbuf[(u + 1) % 2]
                      hA, hB = hpA[u % 2], hpB[u % 2]
                      hA2, hB2 = hpA[(u + 1) % 2], hpB[(u + 1) % 2]
                      psG = ppG.tile([128, 16 * BS], f32, tag="psG")
                      psM = ppM.tile([128, 6 * BS], f32, tag="psM")

                      def hs(h, j):
                          return h[:, (j % 2) * BS:(j % 2) * BS + BS]

                      def mm(pst, lo, kc, mt, rhs, start=False, stop=False):
                          if kc < KH and mt < 16:
                              lhs = wsb8[:, (kc * 16 + mt) * 128:
                                         (kc * 16 + mt + 1) * 128]
                          else:
                              lhs = w_ap(kc, mt)
                          nc.tensor.matmul(
                              pst[:, (mt - lo) * BS:(mt - lo + 1) * BS],
                              lhs, rhs, start=start, stop=stop,
                              skip_group_check=True)

                      # gx+bias inject (identity matmuls, one per PSUM bank)
                      nc.tensor.matmul(psG[:], eye[:],
                                       buf[:, ui * PF:ui * PF + 128],
                                       start=True, stop=False,
                                       skip_group_check=True)
                      nc.tensor.matmul(psM[:], eye[:],
                                       buf[:, ui * PF + 128:(ui + 1) * PF],
                                       start=True, stop=False,
                                       skip_group_check=True)

                      # Block-1: all kc0/kc1 matmuls (need hA only)
                      for kc in (0, 1):
                          rhs = hs(hA, kc)
                          for mt in (16, 17):
                              mm(psM, 16, kc, mt, rhs)
                          for mt in range(0, 8):
                              mm(psG, 0, kc, mt, rhs)
                          for mt in (18, 19):
                              mm(psM, 16, kc, mt, rhs)
                          for mt in range(8, 16):
                              mm(psG, 0, kc, mt, rhs)
                          for mt in (20, 21):
                              mm(psM, 16, kc, mt, rhs)
                      # Block-2 (needs hB): m first -> Gm -> modx on
                      # ACT/DVE while PE sweeps gates-A
                      for kc in (2, 3):
                          for mt in (16, 17):
                              mm(psM, 16, kc, mt, hs(hB, kc), stop=(kc == 3))
                      Gm = scp.tile([128, KX * BS], bf16, tag="Gm")
                      nc.scalar.activation(Gm[:], psM[:, 0:2 * BS], AF.Sigmoid)
                      modx = scp.tile([128, KX * BS], bf16, tag="modx")
                      for kx in range(KX):
                          nc.vector.tensor_mul(
                              modx[:, kx * BS:(kx + 1) * BS],
                              Gm[:, kx * BS:(kx + 1) * BS],
                              xt[:, ds(kx * t_steps * BS + (iv + u) * BS, BS)])
                      # chunk-A matmuls complete first: gates-A, cbar-A + x
                      for kc in (2, 3):
                          rhs = hs(hB, kc)
                          for mt in range(0, 8):
                              mm(psG, 0, kc, mt, rhs, stop=(kc == 3))
                          for mt in (18, 19):
                              mm(psM, 16, kc, mt, rhs)
                      for kx in range(KX):
                          for mt in (18, 19):
                              mm(psM, 16, 4 + kx, mt,
                                 modx[:, kx * BS:(kx + 1) * BS],
                                 stop=(kx == KX - 1))
                      # chunk-A tail (ACT/DVE) — PE continues with B below
                      GA = scp.tile([128, 8 * BS], bf16, tag="GA")
                      nc.scalar.activation(GA[:], psG[:, 0:8 * BS], AF.Sigmoid)
                      SA = scp.tile([128, 2 * BS], f32, tag="SA")
                      nc.scalar.activation(SA[:], psM[:, 2 * BS:4 * BS],
                                           AF.Sigmoid, scale=2.0)
                      uuA = scp.tile([128, 2 * BS], bf16, tag="uuA")
                      wwA = scp.tile([128, 2 * BS], bf16, tag="wwA")
                      ffA = scp.tile([128, 2 * BS], f32, tag="ffA")
                      nc.vector.tensor_sub(uuA[:], GA[:, 0:16], GA[:, 16:32])
                      nc.vector.tensor_mul(wwA[:], GA[:, 32:48], uuA[:])
                      nc.vector.tensor_add(ffA[:], GA[:, 16:32], wwA[:])
                      rA = scp.tile([128, 2 * BS], f32, tag="rA")
                      nc.vector.scalar_tensor_tensor(
                          rA[:], SA[:], -2.0, cprev[:, 0:16], MUL, ADD)
                      tA = scp.tile([128, 2 * BS], f32, tag="tA")
                      nc.vector.tensor_mul(tA[:], ffA[:], rA[:])
                      nc.vector.scalar_tensor_tensor(
                          cnew[:, 0:16], SA[:], 2.0, tA[:], MUL, ADD)
                      S2A = scp.tile([128, 2 * BS], f32, tag="S2A")
                      nc.scalar.activation(S2A[:], cnew[:, 0:2 * BS],
                                           AF.Sigmoid, bias=negtwo[:],
                                           scale=2.0)
                      # hA' = (S2A - 0.5) * oA   (h stored as h/2)
                      nc.vector.scalar_tensor_tensor(
                          hA2[:], S2A[:], -0.5, GA[:, 48:64], ADD, MUL)

                      # chunk-B matmuls: gates-B, cbar-B + x
                      for kc in (2, 3):
                          rhs = hs(hB, kc)
                          for mt in range(8, 16):
                              mm(psG, 0, kc, mt, rhs, stop=(kc == 3))
                          for mt in (20, 21):
                              mm(psM, 16, kc, mt, rhs)
                      for kx in range(KX):
                          for mt in (20, 21):
                              mm(psM, 16, 4 + kx, mt,
                                 modx[:, kx * BS:(kx + 1) * BS],
                                 stop=(kx == KX - 1))
                      # chunk-B tail
                      GB = scp.tile([128, 8 * BS], bf16, tag="GB")
                      nc.scalar.activation(GB[:], psG[:, 8 * BS:16 * BS],
                                           AF.Sigmoid)
                      SB = scp.tile([128, 2 * BS], f32, tag="SB")
                      nc.scalar.activation(SB[:], psM[:, 4 * BS:6 * BS],
                                           AF.Sigmoid, scale=2.0)
                      uuB = scp.tile([128, 2 * BS], bf16, tag="uuB")
                      wwB = scp.tile([128, 2 * BS], bf16, tag="wwB")
                      ffB = scp.tile([128, 2 * BS], f32, tag="ffB")
                      nc.vector.tensor_sub(uuB[:], GB[:, 0:16], GB[:, 16:32])
                      nc.vector.tensor_mul(wwB[:], GB[:, 32:48], uuB[:])
                      nc.vector.tensor_add(ffB[:], GB[:, 16:32], wwB[:])
                      rB = scp.tile([128, 2 * BS], f32, tag="rB")
                      nc.vector.scalar_tensor_tensor(
                          rB[:], SB[:], -2.0, cprev[:, 16:32], MUL, ADD)
                      tB = scp.tile([128, 2 * BS], f32, tag="tB")
                      nc.vector.tensor_mul(tB[:], ffB[:], rB[:])
                      nc.vector.scalar_tensor_tensor(
                          cnew[:, 16:32], SB[:], 2.0, tB[:], MUL, ADD)
                      S2B = scp.tile([128, 2 * BS], f32, tag="S2B")
                      nc.scalar.activation(S2B[:], cnew[:, 2 * BS:4 * BS],
                                           AF.Sigmoid, bias=negtwo[:],
                                           scale=2.0)
                      nc.vector.scalar_tensor_tensor(
                          hB2[:], S2B[:], -0.5, GB[:, 48:64], ADD, MUL)
                      # hist copies on Pool engine (off the DVE chain)
                      nc.gpsimd.tensor_copy(
                          hist[:, ds((iv + u + 1) * KH * BS, 2 * BS)], hA2[:])
                      nc.gpsimd.tensor_copy(
                          hist[:, ds((iv + u + 1) * KH * BS + 2 * BS, 2 * BS)],
                          hB2[:])

                      # refill the just-drained gx half-buffer (4 steps ahead+1)
                      if u % 4 == 3:
                          nc.sync.dma_start(
                              gxb[(u // 4) % 2][:],
                              gx_d[:, ds((iv + u + 5) * PF, half)])

            # ---- output: cast history to fp32 ----
            nc.gpsimd.dma_start(ho_d[:], hist[:, KH * BS:(t_steps + 1) * KH * BS])

    nc.compile()
    return nc


def _pack_inputs(x, h0, c0, W_f_short, b_f_short, W_f_long, b_f_long,
                 W_alpha, b_alpha, W_m, b_m, W_C, b_C, W_o, b_o, t_steps):
    W_all = np.concatenate(
        [W_f_short, W_f_long, W_alpha, W_o, W_m, W_C], axis=1).astype(np.float32)
    b_all = np.concatenate(
        [b_f_short, b_f_long, b_alpha, b_o, b_m, b_C], axis=0).astype(np.float32)
    # permute 128-col blocks to the chunk-major layout
    W_all = W_all.reshape(D + U, MT, 128)[:, MT_PERM].reshape(D + U, WCOL)
    b_all = b_all.reshape(MT, 128)[MT_PERM].reshape(WCOL)
    # h stored as h/2 on device -> h-part weight rows x2 (exact in bf16)
    W_all[:U] *= 2.0
    # Wsb[p, kc*WCOL + m] = W_all[kc*128 + p, m]
    wsb = np.ascontiguousarray(
        W_all.reshape(6, 128, WCOL).transpose(1, 0, 2).reshape(128, 6 * WCOL)
    ).astype(BF16)
    b22 = np.ascontiguousarray(b_all.reshape(MT, 128).T).astype(np.float32)
    bc1 = np.ascontiguousarray(
        np.repeat(b_C.astype(np.float32).reshape(MT_C, 128).T[:, :, None],
                  BS, axis=2).reshape(128, MT_C * BS))
    bc64 = np.tile(bc1, (1, TB)).astype(BF16)
    eye = np.eye(128, dtype=np.float32).astype(BF16)
    wsb8 = np.ascontiguousarray(
        W_all[0:U, 0:2048].reshape(4, 128, 2048).transpose(1, 0, 2)
        .reshape(128, 8192)).astype(ml_dtypes.float8_e4m3fn)

    ins = []
    for i in range(NC):
        xi = np.asarray(x[i * BS:(i + 1) * BS, :t_steps]).astype(np.float32)
        # xt[p, kc*T*BS + t*BS + b] = x[b, t, kc*128 + p]
        xti = np.ascontiguousarray(
            xi.reshape(BS, t_steps, KX, 128).transpose(3, 2, 1, 0)
            .reshape(128, KX * t_steps * BS)).astype(BF16)
        h0i = np.ascontiguousarray(
            (np.asarray(h0[i * BS:(i + 1) * BS]).astype(np.float32) * 0.5)
            .reshape(BS, KH, 128).transpose(2, 1, 0).reshape(128, KH * BS)
        ).astype(BF16)
        c0i = np.ascontiguousarray(
            (np.asarray(c0[i * BS:(i + 1) * BS]).astype(np.float32) + 1.0)
            .reshape(BS, MT_C, 128).transpose(2, 1, 0).reshape(128, MT_C * BS)
        ).astype(np.float32)
        ins.append({"wsb": wsb, "xt": xti, "b22": b22, "bc64": bc64,
                    "eye": eye, "wsb8": wsb8, "h0p": h0i, "c0p": c0i})
    return ins


def kernel(**inputs):
    t_steps = int(np.asarray(inputs["x"]).shape[1])
    if t_steps not in _CACHE:
        _CACHE[t_steps] = _build_program(t_steps)
    nc = _CACHE[t_steps]

    from concourse.bass_utils import run_bass_kernel_spmd
    ins = _pack_inputs(t_steps=t_steps, **inputs)
    res = run_bass_kernel_spmd(nc, ins, core_ids=list(range(NC)))

    out = np.empty((B, t_steps, U), dtype=np.float32)
    for i in range(NC):
        ho = np.asarray(res.results[i]["ho"])  # [128, T*KH*BS]
        a = ho.reshape(128, t_steps, KH, BS)
        # stored h/2 -> rescale by 2 (exact)
        out[i * BS:(i + 1) * BS] = (
            a.transpose(3, 1, 2, 0).reshape(BS, t_steps, U) * 2.0)
    return out


if __name__ == "__main__":
    rng = np.random.default_rng(0)
    sh = {"x": (B, T, D), "h0": (B, U), "c0": (B, U)}
    demo = {k: rng.standard_normal(v).astype(np.float32) * 0.1
            for k, v in sh.items()}
    for n, s in [("W_f_short", (D + U, U)), ("W_f_long", (D + U, U)),
                 ("W_alpha", (D + U, U)), ("W_m", (D + U, D)),
                 ("W_C", (D + U, U)), ("W_o", (D + U, U))]:
        demo[n] = rng.standard_normal(s).astype(np.float32) * 0.05
    for n, s in [("b_f_short", U), ("b_f_long", U), ("b_alpha", U),
                 ("b_m", D), ("b_C", U), ("b_o", U)]:
        demo[n] = np.zeros(s, np.float32)
    out = kernel(**demo)
    print(out.shape, out.dtype)
